# revision 61
# baseline (speedup 1.0000x reference)
"""Causal multi-head attention kernel for Trainium2 (8 NeuronCores).

Problem: B=2, H=16, S=2048, D=64 causal attention (softmax over last axis).
Sharding: 32 (batch, head) pairs split 4-per-core across 8 cores; each core
computes its heads independently (no collectives).

Per-core algorithm (all in the "S-transposed" layout so no transposes of the
probability matrix are ever needed):
  - Host pre-packs, per core:
      qt  [4, 65, 2048] bf16  : per head, Q^T (d-major) + a 65th row of 1.0
      kt  [4, 65, 2048] bf16  : per head, K^T * (128*log2e/8) + a 65th row
                                of 16256.0 -- the QK matmul then directly
                                yields y2' = s*16*log2e + 127*128, i.e. the
                                score expressed as a biased bf16 exponent in
                                lsb units (the bias rides the extra
                                contraction row, exact in f32 accumulation)
      vg  [4, 128, 16, 65] bf16: per head, V tiles [128, 65] with a ones
                                column appended (col 64) -> PV matmul also
                                produces the softmax row-sum for free
      tri [128, 128] bf16     : tri[k, c] = 1 if c >= k else 0 (intra-block
                                causal keep-mask in S^T coords)
  - For each head, for each k-block kb (128 keys):
      S^T strip [k=128, q=kb*128..2047] = K_blk^T.T @ Q^T  (PE, bf16),
      written into a rotation of five 512-col PSUM strip slots.
      P^T = exp(s/8) per piece, on the PIECE'S SLOT'S DEDICATED ENGINE
      (SLOT_ENG, 3 ACT : 2 DVE interleaved):
        * ACT slots: activation Exp (scale=1/(16*128*log2e) after the
          pre-scaling, bias removing the 16256 offset), PSUM->SBUF bf16
        * DVE slots: one 8-op custom DVE instruction (EXP2_BITS_ATT)
          computes i16 = trunc(y2' + c_abs*|f| + c_b), f = y2'-round128(y2')
          -- those int16 bits ARE bfloat16(2^((y2'-16256)/128)) ~ exp(s/8)
          with ~0.6% rms element error (|f| corrects the Schraudolph interp
          kink; zero-mean so it cancels further under normalization).  The
          int16 tile is bitcast to bf16 for downstream use.
      Engine-dedicated slots keep the two exp WAR chains decoupled; five
      slots give the rotation enough depth that slot-reuse sem latency
      stays off the critical path (measured fastest on HW).
      diagonal 128x128 block masked via elementwise tri multiply (DVE;
      NEVER gpsimd/Pool -- Q7 software ops cost ~1us each on HW)
      O accumulation: for each 128-row q block, acc[q] += P^T_chunk.T @ V_blk
        (PE, accumulating in PSUM; 16 accumulators packed 7/7/2 per bank,
         one has_written "zero region" start per bank)
  - Normalize per acc bank: DVE copies the bank to SBUF, DVE reciprocal of
    col 64 + per-partition scalar multiply, batched f32 DMA out (SP queue).

kernel(Q, K, V, mask) takes the full unsharded fp32 inputs and returns the
full [2, 16, 2048, 64] fp32 output.
"""

import math
import sys

if "/opt/trn_rl_repo" not in sys.path:
    sys.path.insert(0, "/opt/trn_rl_repo")

import numpy as np
import ml_dtypes

B, H, S, D = 2, 16, 2048, 64
N_CORES = 8
HEADS_PER_CORE = (B * H) // N_CORES  # 4
KB = S // 128  # 16 k-blocks per head
QS = S // 128  # 16 q-subblocks per head

_BF16 = ml_dtypes.bfloat16

# accumulator bank packing: q_subs 0-6 -> bank A, 7-13 -> bank B, 14-15 -> C
_BANK_FIRST = (0, 7, 14)   # first q_sub written in each accumulator bank
_BANK_LAST = (6, 13, 15)   # last q_sub written in each accumulator bank

_built = {}
MODE = "full"  # full | qk_only | qk_exp (timing ablations)
ST_BUFS = 2  # PSUM strip-tile slots

# exp2-via-int16-bits (Schraudolph + tent correction): the host scales K^T by
# EXP2_A and appends a 65th contraction row (qt=1, kt=16256) so the QK matmul
# yields y2' = s * 128*log2e/8 + 127*128 -- the score already expressed as a
# biased bf16-exponent in lsb units.  A single 8-op custom DVE op computes
#   i16 = trunc(y2' + c_abs*|f| + c_b),   f = y2' - round128(y2')
# whose int16 bits are bfloat16(2^((y2'-16256)/128)) ~ exp(s/8) with ~0.6%
# rms element error (the |f| term corrects the interp kink of the bit trick;
# the error is zero-mean so it largely cancels under softmax normalization).
EXP2_A = 128.0 * math.log2(math.e) / 8.0
EXP2_BIAS = 16256.0             # 127<<7, exact in bf16
ACT_SCALE = 0.125 / EXP2_A      # ACT path reads the same pre-scaled scores
ACT_BIAS = -EXP2_BIAS * ACT_SCALE
CBIG = 1.5 * 2.0 ** 30          # f32 ulp=128 -> +/- CBIG rounds to mult of 128
C_ABS = -0.167875
C_B = -1.96298 + 0.5            # fit constant + trunc-to-round compensation
C_DIAG_B = -4.2133 + 0.5        # zero-mean constant for the uncorrected trick
# share of exp ELEMENTS routed to the DVE exp2 path (per mille)
DVE_SHARE = 480
# engine for the diagonal-block tri mask multiply: "pool" or "dve".
# NOTE: gpsimd/Pool tensor ops are SOFTWARE on Q7 DSPs and cost ~1us each on
# real HW regardless of size -- never route per-strip compute there.
TRI_ENGINE = "dve"
# engine pair for normalize: DVE copies acc->SBUF, Pool does recip-multiply.
# Same Q7 cost trap as TRI_ENGINE: keep False.
NORM_POOL = False
# engine for the acc-bank PSUM->SBUF copy: "act" offloads it to ACT's slack
# ("act" measured worse: ACT reloads its activation table between Copy/Exp)
COPY_ENGINE = "dve"
# batch the per-bank normalize into one strided reciprocal (+ broadcast
# multiply).  Measured SLOWER on HW both ways (+17us) -- strided/stride-0
# DVE APs lose the fast path; keep False.
NORM_BATCH = False
# buffers per pt destination tag (pa/pd)
PT_BUFS = 4
# PSUM strip slot widths (f32 cols); total <= 5 banks (2560 f32), acc uses 3.
# SLOT_ENG dedicates each slot to one exp engine ("a" = ACT exp activation,
# "d" = DVE exp2-bits custom op) so the two WAR chains never share a slot.
# 5x512 with DVE slots interleaved 2-of-5 measured fastest on HW.
SLOT_W = (512, 512, 512, 512, 512)
SLOT_ENG = ("a", "a", "d", "a", "d")

_EXP2_OP = None
_EXP2_DIAG_OP = None


def _exp2_diag_ref(in0, in1, s0, s1, imm2):
    """Numpy reference for EXP2_DIAG_ATT: uncorrected exp2-bits with the
    causal keep-mask (col index >= per-partition k) fused in."""
    x = np.asarray(in0, np.float32)
    idx = np.arange(x.shape[-1], dtype=np.float32)[None, :]
    thr = np.asarray(s1, np.float32)
    out = (x + np.float32(s0)) * (idx >= thr)
    return out.astype(np.float32)


def _register_exp2_diag_op():
    """Causal-diagonal variant: mask folded into the exp via Idx."""
    global _EXP2_DIAG_OP
    if _EXP2_DIAG_OP is not None:
        return _EXP2_DIAG_OP
    from concourse import dve_ops
    from concourse.dve_spec import Spec, Src0, C0, C1, Idx, lower, _has_src1
    from concourse.dve_uop import DveOpSpec

    name = "EXP2_DIAG_ATT"
    for op in dve_ops.OPS:
        if op.name == name:
            _EXP2_DIAG_OP = op
            return op
    body = (Src0 + C0) * (Idx >= C1)
    spec = Spec(body=body, reference=_exp2_diag_ref)
    op = dve_ops.DveOp(name, spec, subdim=False, uops_sha={})
    dve_ops.OPS.append(op)
    dve_ops._SUB_OPCODE_FOR_NAME[name] = (
        dve_ops._CUSTOM_DVE_ROW_BASE + len(dve_ops.OPS) - 1)
    dve_ops.CUSTOM_DVE_SPECS[name] = spec
    for ver in ("v3", "v4"):
        try:
            compiled = DveOpSpec(
                name=name,
                opcode=dve_ops.get_dve_sub_opcode(name),
                uops=lower(spec, ver=ver),
                rd1_en=_has_src1(spec),
            )
            op.uops_sha[ver] = compiled.sha(ver)
        except Exception:
            pass
    _EXP2_DIAG_OP = op
    return op


def _exp2_ref(in0, in1, s0, s1, imm2):
    """Numpy reference for the EXP2_BITS custom DVE op (f32-faithful)."""
    x = np.asarray(in0, np.float32)
    yb = (x + np.float32(imm2)).astype(np.float32)
    k = (yb - np.float32(imm2)).astype(np.float32)
    f = (x - k).astype(np.float32)
    out = x + (np.abs(f) * np.float32(s0) + np.float32(s1))
    return out.astype(np.float32)


def _register_exp2_op():
    """Define + register the custom DVE op (idempotent)."""
    global _EXP2_OP
    if _EXP2_OP is not None:
        return _EXP2_OP
    from concourse import dve_ops
    from concourse.dve_spec import Spec, Src0, Src1, C0, C1, C2, Zero, maxx, sq
    from concourse.dve_spec import lower, _has_src1
    from concourse.dve_uop import DveOpSpec

    name = "EXP2_BITS_ATT"
    for op in dve_ops.OPS:
        if op.name == name:
            _EXP2_OP = op
            return op

    k = (Src0 + C2) - C2
    f = Src0 - k
    body = Src0 + (maxx(f, Zero - f) * C0 + C1)
    spec = Spec(body=body, reference=_exp2_ref)
    op = dve_ops.DveOp(name, spec, subdim=False, uops_sha={})
    dve_ops.OPS.append(op)
    dve_ops._SUB_OPCODE_FOR_NAME[name] = (
        dve_ops._CUSTOM_DVE_ROW_BASE + len(dve_ops.OPS) - 1)
    dve_ops.CUSTOM_DVE_SPECS[name] = spec
    for ver in ("v3", "v4"):
        try:
            compiled = DveOpSpec(
                name=name,
                opcode=dve_ops.get_dve_sub_opcode(name),
                uops=lower(spec, ver=ver),
                rd1_en=_has_src1(spec),
            )
            op.uops_sha[ver] = compiled.sha(ver)
        except Exception:
            pass
    _EXP2_OP = op
    return op


def _emit(tc, nc, mybir, qt, kt, vg, tri, o, causal, reps=1):
    from contextlib import ExitStack

    f32 = mybir.dt.float32
    bf = mybir.dt.bfloat16
    i16 = mybir.dt.int16
    Exp = mybir.ActivationFunctionType.Exp
    Mult = mybir.AluOpType.mult
    Add = mybir.AluOpType.add

    with ExitStack() as ctx:
        const = ctx.enter_context(tc.tile_pool(name="const", bufs=1))
        qk = ctx.enter_context(tc.tile_pool(name="qk", bufs=2))
        vpool = ctx.enter_context(tc.tile_pool(name="vp", bufs=2))
        ptp = ctx.enter_context(tc.tile_pool(name="ptp", bufs=PT_BUFS))
        outp = ctx.enter_context(tc.tile_pool(name="outp", bufs=4))
        cpool = ctx.enter_context(tc.tile_pool(name="cpool", bufs=4))
        stp = ctx.enter_context(tc.tile_pool(name="stp", bufs=1, space="PSUM"))
        accp = ctx.enter_context(tc.tile_pool(name="accp", bufs=1, space="PSUM"))

        tri_t = const.tile([128, 128], bf, name="tri_t")
        nc.sync.dma_start(tri_t[:, :], tri[:, :])

        exp2_op = _register_exp2_op()
        exp2_diag_op = _register_exp2_diag_op()
        abias = const.tile([128, 1], f32, name="abias")
        nc.vector.memset(abias[:, :], ACT_BIAS)
        iot_i = const.tile([128, 1], mybir.dt.int32, name="iot_i")
        nc.gpsimd.iota(iot_i[:, :], [[0, 1]], base=0, channel_multiplier=1)
        iot = const.tile([128, 1], f32, name="iot")
        nc.vector.tensor_copy(iot[:, :], iot_i[:, :])

        # dummy exp issued first: walrus places the ~2.7us ACT table load
        # before the first ACTIVATE in the stream, so doing one on a tiny
        # constant tile overlaps the table load with the input DMAs instead
        # of serializing it before the first real exp
        warm = const.tile([128, 1], f32, name="warm")
        nc.vector.memset(warm[:, :], 0.0)
        nc.scalar.activation(warm[:, :], warm[:, :], Exp)

        from contextlib import nullcontext
        with (tc.For_i(0, reps, 1) if reps > 1 else nullcontext()):
          rep = 0  # body emitted once; hardware loop repeats it

          def load_head(hh, staged=False):
              """DMA head hh's qt/kt/vg into fresh tiles (qt on the SP HWDGE
              queue, kt/vg on the gpsimd SWDGE queue so they load in
              parallel; leading chunks unblock the first QK early)."""
              qt_t = qk.tile([65, S], bf, tag="qt", name=f"qt_{rep}_{hh}")
              kt_t = qk.tile([65, S], bf, tag="kt", name=f"kt_{rep}_{hh}")
              vg_t = vpool.tile([128, KB, 65], bf, tag="vg",
                                name=f"vg_{rep}_{hh}")
              if staged:
                  nc.gpsimd.dma_start(kt_t[:, :128], kt[hh][:, :128])
                  nc.sync.dma_start(qt_t[:, :1024], qt[hh][:, :1024])
                  nc.gpsimd.dma_start(kt_t[:, 128:], kt[hh][:, 128:])
                  nc.sync.dma_start(qt_t[:, 1024:], qt[hh][:, 1024:])
              else:
                  nc.gpsimd.dma_start(kt_t[:, :], kt[hh][:, :])
                  nc.sync.dma_start(qt_t[:, :], qt[hh][:, :])
              nc.gpsimd.dma_start(vg_t[:, :, :], vg[hh])
              return qt_t, kt_t, vg_t

          next_tiles = load_head(0, staged=True)
          for h in range(HEADS_PER_CORE):
                  qt_t, kt_t, vg_t = next_tiles

                  accA = accp.tile([128, 7, 65], f32, tag="accA", name=f"accA_{rep}_{h}")
                  accB = accp.tile([128, 7, 65], f32, tag="accB", name=f"accB_{rep}_{h}")
                  accC = accp.tile([128, 2, 65], f32, tag="accC", name=f"accC_{rep}_{h}")

                  def acc(i):
                      if i < 7:
                          return accA[:, i, :]
                      if i < 14:
                          return accB[:, i - 7, :]
                      return accC[:, i - 14, :]

                  def strip_pieces(kb):
                      """Split strip kb into pieces following the global slot
                      rotation (widths bounded by each slot's size).  With
                      equal-width slots the diagonal (first) piece is swapped
                      onto a DVE slot when possible so the causal mask can be
                      fused into its exp2 op (no separate tri-multiply)."""
                      nonlocal slot_plan_idx
                      q0 = 128 * kb if causal else 0
                      cols = S - q0
                      pieces = []
                      hs = 0
                      i = slot_plan_idx
                      while hs < cols:
                          w = min(SLOT_W[i % len(SLOT_W)], cols - hs)
                          pieces.append([q0, hs, w, i % len(SLOT_W)])
                          hs += w
                          i += 1
                      return [tuple(p) for p in pieces]

                  def emit_qk(kb, pieces):
                      """QK matmuls for strip kb; returns the st tiles."""
                      nonlocal slot_plan_idx
                      sts = []
                      for q0, hs, hw, sl in pieces:
                          st = stp.tile([128, SLOT_W[sl]], f32, tag=f"st{sl}",
                                        name=f"st_{rep}_{h}_{kb}_{hs}")
                          for c0 in range(0, hw, 512):
                              cw = min(512, hw - c0)
                              nc.tensor.matmul(
                                  st[:, c0:c0 + cw],
                                  lhsT=kt_t[:, kb * 128:(kb + 1) * 128],
                                  rhs=qt_t[:, q0 + hs + c0:q0 + hs + c0 + cw],
                                  start=True, stop=True,
                              )
                          sts.append(st)
                          slot_plan_idx += 1
                      return sts

                  def normalize_bank(qs_lo, qs_hi):
                      """acc bank -> SBUF copy (DVE), per-q_sub Pool
                      normalize_recip, one batched DMA out (rows r of out
                      tile j map to q = qs*128+r)."""
                      n = qs_hi - qs_lo
                      if qs_lo == 0:
                          bank = accA
                      elif qs_lo == 7:
                          bank = accB
                      else:
                          bank = accC
                      cp = cpool.tile([128, n, 65], f32, tag=f"cp{qs_lo}",
                                      name=f"cp_{rep}_{h}_{qs_lo}")
                      if COPY_ENGINE == "act":
                          nc.scalar.activation(
                              cp[:, :, :], bank[:, :, :],
                              mybir.ActivationFunctionType.Copy)
                      else:
                          nc.vector.tensor_copy(cp[:, :, :], bank[:, :, :])
                      ot = outp.tile([128, n, 64], f32, tag=f"ot{qs_lo}",
                                     name=f"ot_{rep}_{h}_{qs_lo}")
                      if NORM_BATCH:
                          # one strided reciprocal over all n denominators
                          # (the stride-0 broadcast multiply measured slow on
                          # HW; keep the multiply per q_sub)
                          nc.vector.reciprocal(cp[:, :, 64:65],
                                               cp[:, :, 64:65])
                          for j in range(n):
                              nc.vector.tensor_scalar_mul(
                                  ot[:, j, :], cp[:, j, :64], cp[:, j, 64:65])
                      else:
                          for j in range(n):
                              nc.vector.reciprocal(cp[:, j, 64:65],
                                                   cp[:, j, 64:65])
                              nc.vector.tensor_scalar_mul(
                                  ot[:, j, :], cp[:, j, :64], cp[:, j, 64:65])
                      dst = o[h, qs_lo * 128:qs_hi * 128, :].rearrange(
                          "(j r) c -> r j c", r=128)
                      nc.sync.dma_start(dst, ot[:, :, :])

                  slot_plan_idx = 0

                  cur_pieces = strip_pieces(0)
                  sts = emit_qk(0, cur_pieces)
                  for kb in range(KB):
                      # exp of strip kb: each piece runs on its slot's
                      # dedicated engine (ACT exp or DVE exp2-bits custom op)
                      # so the per-slot WAR chains never mix engines
                      pieces = cur_pieces
                      pts = []
                      for (q0, hs, hw, sl), st in zip(pieces, sts):
                          if MODE == "qk_only":
                              continue
                          diag = causal and hs == 0
                          if SLOT_ENG[sl] == "d":
                              pti = ptp.tile([128, 1024], i16, tag="pd",
                                             name=f"pd_{rep}_{h}_{kb}_{hs}")
                              if diag and hw <= 256:
                                  # short diagonal piece: causal keep-mask
                                  # fused into the (uncorrected) exp2 op
                                  nc.vector._custom_dve(
                                      exp2_diag_op, out=pti[:, :hw],
                                      in0=st[:, :hw],
                                      s0=C_DIAG_B, s1=iot[:, :], imm2=0.0)
                                  diag = False
                              else:
                                  nc.vector._custom_dve(
                                      exp2_op, out=pti[:, :hw],
                                      in0=st[:, :hw],
                                      s0=C_ABS, s1=C_B, imm2=CBIG)
                              pt = pti.bitcast(bf)
                          else:
                              pt = ptp.tile([128, 1024], bf, tag="pa",
                                            name=f"pa_{rep}_{h}_{kb}_{hs}")
                              nc.scalar.activation(pt[:, :hw], st[:, :hw],
                                                   Exp, scale=ACT_SCALE,
                                                   bias=abias[:, :])
                          if diag:
                              nc.vector.tensor_mul(pt[:, :128],
                                                   pt[:, :128],
                                                   tri_t[:, :])
                          pts.append(pt)
                      # QK for strip kb+1 goes to PE before PV of strip kb so
                      # the PE never stalls behind ACT
                      if kb + 1 < KB:
                          cur_pieces = strip_pieces(kb + 1)
                          sts = emit_qk(kb + 1, cur_pieces)
                      # prefetch the next head's inputs early in this head
                      if kb == 1 and h + 1 < HEADS_PER_CORE:
                          next_tiles = load_head(h + 1)
                      # PV accumulation for strip kb
                      if MODE != "full":
                          continue
                      for (q0, hs, hw, sl), pt in zip(pieces, pts):
                          qs_range = list(range((q0 + hs) // 128,
                                                (q0 + hs + hw) // 128))
                          # the diagonal q_sub (== kb) additionally depends on
                          # the tri-multiply; emit it last so the PE can
                          # start the other PV matmuls as soon as exp is done.
                          # (at kb==0 keep ascending order: the bank-group
                          # start=True matmuls must be first into each bank)
                          if causal and kb > 0 and qs_range and qs_range[0] == kb:
                              qs_range = qs_range[1:] + [kb]
                          for q_sub in qs_range:
                              m = q_sub * 128 - q0 - hs
                              last_kb = q_sub if causal else KB - 1
                              nc.tensor.matmul(
                                  acc(q_sub),
                                  lhsT=pt[:, m:m + 128],
                                  rhs=vg_t[:, kb, :],
                                  start=(kb == 0 and q_sub in _BANK_FIRST),
                                  stop=(q_sub in _BANK_LAST and kb == last_kb),
                              )
                      # normalize accumulator banks as soon as they complete
                      if causal:
                          if kb == 6:
                              normalize_bank(0, 7)
                          elif kb == 13:
                              normalize_bank(7, 14)
                          elif kb == 15:
                              normalize_bank(14, 16)
                  if not causal:
                      normalize_bank(0, 7)
                      normalize_bank(7, 14)
                      normalize_bank(14, 16)


def build_nc(causal=True, reps=1):
    """Build + compile the per-core Bass program (cached)."""
    key = ("nc", causal, reps, MODE, DVE_SHARE, TRI_ENGINE, NORM_POOL,
           C_ABS, C_B, SLOT_W, SLOT_ENG, COPY_ENGINE, NORM_BATCH, PT_BUFS)
    if key in _built:
        return _built[key]
    import concourse.bacc as bacc
    from concourse import mybir, tile

    nc = bacc.Bacc("TRN2", target_bir_lowering=False, debug=False,
                   num_devices=N_CORES)
    qt = nc.dram_tensor("qt", (HEADS_PER_CORE, 65, S),
                        mybir.dt.bfloat16, kind="ExternalInput").ap()
    kt = nc.dram_tensor("kt", (HEADS_PER_CORE, 65, S),
                        mybir.dt.bfloat16, kind="ExternalInput").ap()
    vg = nc.dram_tensor("vg", (HEADS_PER_CORE, 128, KB, 65),
                        mybir.dt.bfloat16, kind="ExternalInput").ap()
    tri = nc.dram_tensor("tri", (128, 128), mybir.dt.bfloat16,
                         kind="ExternalInput").ap()
    o = nc.dram_tensor("o", (HEADS_PER_CORE, S, D), mybir.dt.float32,
                       kind="ExternalOutput").ap()
    with tile.TileContext(nc) as tc:
        _emit(tc, nc, mybir, qt, kt, vg, tri, o, causal, reps)
    nc.compile()
    _built[key] = nc
    return nc


def prep_inputs(Q, K, V):
    """Host-side shard + layout prep. Returns list of 8 per-core input dicts."""
    Qf = np.ascontiguousarray(Q, dtype=np.float32).reshape(B * H, S, D)
    Kf = np.ascontiguousarray(K, dtype=np.float32).reshape(B * H, S, D)
    Vf = np.ascontiguousarray(V, dtype=np.float32).reshape(B * H, S, D)

    # [BH, S, D] -> transposed, bf16: [BH, D, S].  K^T is pre-scaled by
    # EXP2_A so QK scores arrive in bf16-exponent lsb units, and both get a
    # 65th contraction row (qt=1, kt=16256) that adds the exponent bias
    # inside the matmul (see EXP2_A / EXP2_BIAS).
    Qt = np.ascontiguousarray(Qf.transpose(0, 2, 1)).astype(_BF16)
    Kt = np.ascontiguousarray(Kf.transpose(0, 2, 1) * np.float32(EXP2_A)
                              ).astype(_BF16)

    # V augmented with ones column, partition-major: [BH, 128, KB, 65]
    Vb = Vf.astype(_BF16)
    vg_all = np.empty((B * H, 128, KB, 65), dtype=_BF16)
    # V[h, kb*128 + r, c] -> vg[h, r, kb, c]
    vg_all[:, :, :, :64] = Vb.reshape(B * H, KB, 128, D).transpose(0, 2, 1, 3)
    vg_all[:, :, :, 64] = _BF16(1.0)

    tri_np = (np.tril(np.ones((128, 128), dtype=np.float32))
              .T.astype(_BF16))  # tri[k, c] = 1 if c >= k
    tri_np = np.ascontiguousarray(tri_np)

    in_maps = []
    for c in range(N_CORES):
        h0 = c * HEADS_PER_CORE
        qt_c = np.empty((HEADS_PER_CORE, 65, S), dtype=_BF16)
        kt_c = np.empty((HEADS_PER_CORE, 65, S), dtype=_BF16)
        for hh in range(HEADS_PER_CORE):
            qt_c[hh, :64] = Qt[h0 + hh]
            qt_c[hh, 64] = _BF16(1.0)
            kt_c[hh, :64] = Kt[h0 + hh]
            kt_c[hh, 64] = _BF16(EXP2_BIAS)
        in_maps.append({
            "qt": qt_c,
            "kt": kt_c,
            "vg": np.ascontiguousarray(vg_all[h0:h0 + HEADS_PER_CORE]),
            "tri": tri_np,
        })
    return in_maps


def _classify_mask(mask):
    m = np.asarray(mask).reshape(S, S)
    if not m.any():
        return "dense"
    if np.array_equal(m, np.triu(np.ones((S, S), dtype=bool), k=1)):
        return "causal"
    raise NotImplementedError("only causal or all-False masks supported")


def run_cores(in_maps, causal=True, reps=1, **kwargs):
    from concourse import bass_utils

    nc = build_nc(causal, reps)
    return bass_utils.run_bass_kernel_spmd(
        nc, in_maps, core_ids=list(range(N_CORES)), **kwargs
    )


def kernel(Q, K, V, mask):
    kind = _classify_mask(mask)
    in_maps = prep_inputs(Q, K, V)
    res = run_cores(in_maps, causal=(kind == "causal"))
    out = np.concatenate([r["o"] for r in res.results], axis=0)
    return out.reshape(B, H, S, D).astype(np.float32)


if __name__ == "__main__":
    rng = np.random.default_rng(0)
    Q = rng.standard_normal((B, H, S, D), dtype=np.float32)
    K = rng.standard_normal((B, H, S, D), dtype=np.float32)
    V = rng.standard_normal((B, H, S, D), dtype=np.float32)
    mask = np.triu(np.ones((S, S), dtype=bool), k=1)[None, None]
    out = kernel(Q, K, V, mask)
    print("out", out.shape, out.dtype)
